# revision 24
# baseline (speedup 1.0000x reference)
"""Bahdanau attention TRN2 Bass kernel.

Full inputs (B=64) are sharded batch-wise across 8 NeuronCores (8 batches
per core); params (Wq, We) are replicated. Each core computes, for its
batches b:
    q_proj = query[b] @ Wq^T                         # [1, 1024]
    energy[k] = sum_h We[h] * tanh(q_proj[h] + key[b,k,h])
    attn = softmax(energy masked)                    # [1, 2048]
    context = attn @ value[b]                        # [1, 1024]

Per-core layout: keys/values stream as [128(k) x 2 x 1024(h)] double tiles
(1 MiB per DMA; keys on the sync HWDGE ring, values on the scalar ring).
 - DVE: X = key + q_bcast (tensor_add), then fused multiply+accumulate
   (scalar_tensor_tensor) of tanh(X) * We_bcast -> energy column [128,1].
 - ACT: tanh / exp.
 - PE : q-projection matmuls, q/recip partition broadcasts, cross-partition
   sums (matmul with ones), attn transpose, context matvec PSUM accumulation.
Softmax skips max-subtraction: energy = sum_h We[h]*tanh(.) with
|We|~N(0,1/H) is O(1), far from fp32 exp overflow.
"""

import os
import sys

import numpy as np

for _p in ("/opt/trn_rl_repo", "/root/.axon_site/_ro/trn_rl_repo"):
    if os.path.isdir(_p) and _p not in sys.path:
        sys.path.insert(0, _p)

import concourse.bacc as bacc  # noqa: E402
import concourse.bass as bass  # noqa: E402
import concourse.tile as tile  # noqa: E402
from concourse import masks, mybir  # noqa: E402
from contextlib import ExitStack  # noqa: E402

F32 = mybir.dt.float32
I32 = mybir.dt.int32
AF = mybir.ActivationFunctionType
OP = mybir.AluOpType

N_CORES = 8
B_TOTAL = 64
B = B_TOTAL // N_CORES  # batches per core
KLEN = 2048
H = 1024
P = 128
NT = KLEN // P  # 16 key/value tiles per batch
KW = 4  # k-tiles per DMA (2 MiB transfers)
ND = NT // KW  # DMAs per batch per stream
NEG_BIG = 1.0e30


def build_bass():
    nc = bacc.Bacc(name="bahdanau")

    query = nc.dram_tensor("query", [B, 1, H], F32, kind="ExternalInput").ap()
    key = nc.dram_tensor("key", [B, KLEN, H], F32, kind="ExternalInput").ap()
    value = nc.dram_tensor("value", [B, KLEN, H], F32, kind="ExternalInput").ap()
    mask = nc.dram_tensor("mask", [B, 1, KLEN], I32, kind="ExternalInput").ap()
    Wq = nc.dram_tensor("Wq", [H, H], F32, kind="ExternalInput").ap()
    We = nc.dram_tensor("We", [H], F32, kind="ExternalInput").ap()
    ctx_out = nc.dram_tensor("context", [B, 1, H], F32, kind="ExternalOutput").ap()
    attn_out = nc.dram_tensor("attn", [B, 1, KLEN], F32, kind="ExternalOutput").ap()

    with tile.TileContext(nc) as tc, ExitStack() as ctx:
        singles = ctx.enter_context(tc.tile_pool(name="singles", bufs=1))

        ident = singles.tile([P, P], F32)
        masks.make_identity(nc, ident[:])
        ones_col = singles.tile([P, 1], F32)
        nc.vector.memset(ones_col[:], 1.0)
        ones_row = singles.tile([1, P], F32)
        nc.vector.memset(ones_row[:], 1.0)

        # We broadcast across partitions: [128, 1024]
        we_b = singles.tile([P, H], F32)
        nc.gpsimd.dma_start(out=we_b[:], in_=We.partition_broadcast(P))

        # Persistent per-batch results
        energy = singles.tile([P, B, NT], F32)
        e_all = singles.tile([P, B, NT], F32)  # exp(masked energy)
        recip_all = singles.tile([1, B], F32)  # 1/sum per batch
        q_sb = singles.tile([B, H], F32)  # q_proj rows

        # ---- main pools (allocated below setup pools on the SBUF stack so
        # the batch loop's DMAs don't serialize on the setup pools' release)
        kpool = ctx.enter_context(tc.tile_pool(name="kpool", bufs=2))
        vpool = ctx.enter_context(tc.tile_pool(name="vpool", bufs=2))
        smpool = ctx.enter_context(tc.tile_pool(name="smpool", bufs=2))
        q_bc_all = singles.tile([P, B, H], F32)  # q_proj broadcast per batch
        psum_sm = ctx.enter_context(tc.tile_pool(name="psum_sm", bufs=1, space="PSUM"))
        psum_ctx = ctx.enter_context(
            tc.tile_pool(name="psum_ctx", bufs=1, space="PSUM")
        )

        # ---- setup: q_proj = query @ Wq^T (scoped pools, freed after) ----
        with ExitStack() as setup:
            sp = setup.enter_context(tc.tile_pool(name="setup_sb", bufs=1))
            wqt_pool = setup.enter_context(tc.tile_pool(name="setup_wqt", bufs=2))
            spsum = setup.enter_context(
                tc.tile_pool(name="setup_ps", bufs=3, space="PSUM")
            )
            qps_pool = setup.enter_context(
                tc.tile_pool(name="setup_qps", bufs=1, space="PSUM")
            )

            # load query rows [8, 1024] and transpose to qT [128(d), 64]
            q_nat = sp.tile([B, H], F32)
            nc.sync.dma_start(out=q_nat[:], in_=query[:, 0, :])
            qT = sp.tile([P, B * (H // P)], F32)  # block i at cols [i*8,(i+1)*8)
            for i in range(H // P):
                pt = spsum.tile([P, B], F32, tag="tp")
                nc.tensor.transpose(pt[:], q_nat[:, i * P : (i + 1) * P], ident[:B, :B])
                nc.vector.tensor_copy(qT[:, i * B : (i + 1) * B], pt[:])

            # load Wq natural tiles [128(h'), j, 1024(d)]
            wq_nat = sp.tile([P, H // P, H], F32)
            nc.sync.dma_start(
                out=wq_nat[:],
                in_=Wq.rearrange("(j p) d -> p j d", p=P),
            )
            # interleave per-d-block transposes with the q_proj accumulation
            # so the PE chain is transpose(i) -> matmul(i) -> transpose(i+1)...
            q_ps_tile = qps_pool.tile([B, H], F32, tag="qps")
            q_ps = [q_ps_tile[:, c * 512 : (c + 1) * 512] for c in range(2)]
            for i in range(H // P):
                wqT_i = wqt_pool.tile([P, H], F32)  # [128(d), 1024(h')]
                for j in range(H // P):
                    pt = spsum.tile([P, P], F32, tag="tp")
                    nc.tensor.transpose(
                        pt[:], wq_nat[:, j, i * P : (i + 1) * P], ident[:]
                    )
                    nc.vector.tensor_copy(wqT_i[:, j * P : (j + 1) * P], pt[:])
                for c in range(2):
                    nc.tensor.matmul(
                        q_ps[c],
                        lhsT=qT[:, i * B : (i + 1) * B],
                        rhs=wqT_i[:, c * 512 : (c + 1) * 512],
                        start=(i == 0),
                        stop=(i == H // P - 1),
                        skip_group_check=True,
                    )
            for c in range(2):
                nc.scalar.copy(q_sb[:, c * 512 : (c + 1) * 512], q_ps[c])

        # broadcast every batch's q_proj row across 128 partitions upfront so
        # the per-batch adds never wait behind context matmuls on PE.
        # (rows first moved to a partition-0 tile; matmul operands must be
        # based at partition 0/32/64)
        for b in range(B):
            q_row = smpool.tile([1, H], F32, tag="qrow")
            nc.gpsimd.dma_start(out=q_row[:], in_=q_sb[b : b + 1, :])
            for c in range(2):
                qps = psum_sm.tile([P, 512], F32, tag="sm")
                nc.tensor.matmul(
                    qps[:],
                    lhsT=ones_row[:],
                    rhs=q_row[:, c * 512 : (c + 1) * 512],
                    start=True,
                    stop=True,
                )
                nc.scalar.copy(q_bc_all[:, b, c * 512 : (c + 1) * 512], qps[:])

        for b in range(B):
            q_bc = q_bc_all[:, b, :]
            # --- energies over key tiles (KW k-tiles per 2 MiB DMA) ---
            for d in range(ND):
                kt = kpool.tile([P, KW, H], F32)
                nc.sync.dma_start(
                    out=kt[:],
                    in_=key[b, d * KW * P : (d + 1) * KW * P, :].rearrange(
                        "(w p) h -> p w h", p=P
                    ),
                )
                for j in range(KW):
                    nc.vector.tensor_add(kt[:, j, :], kt[:, j, :], q_bc)
                nc.scalar.activation(kt[:], kt[:], AF.Tanh)
                for j in range(KW):
                    t = KW * d + j
                    nc.vector.scalar_tensor_tensor(
                        out=kt[:, j, :],
                        in0=kt[:, j, :],
                        scalar=1.0,
                        in1=we_b[:],
                        op0=OP.mult,
                        op1=OP.mult,
                        accum_out=energy[:, b, t : t + 1],
                    )

            # --- mask + exp ---
            mt_i = smpool.tile([P, NT], I32, tag="mi")
            nc.gpsimd.dma_start(
                out=mt_i[:], in_=mask[b, 0, :].rearrange("(t p) -> p t", p=P)
            )
            mt_f = smpool.tile([P, NT], F32, tag="mf")
            nc.vector.tensor_copy(mt_f[:], mt_i[:])
            pen = smpool.tile([P, NT], F32, tag="pen")
            # (mask - 1) * BIG : 0 where mask==1, -BIG where mask==0
            nc.vector.tensor_scalar(
                out=pen[:], in0=mt_f[:], scalar1=-1.0, scalar2=NEG_BIG,
                op0=OP.add, op1=OP.mult,
            )
            me = smpool.tile([P, NT], F32, tag="me")
            nc.vector.tensor_mul(me[:], energy[:, b, :], mt_f[:])
            nc.vector.tensor_add(me[:], me[:], pen[:])
            nc.scalar.activation(e_all[:, b, :], me[:], AF.Exp)

            # --- softmax denominator via PE column sums ---
            ps16 = psum_sm.tile([NT, 1], F32, tag="sm")
            nc.tensor.matmul(
                ps16[:], lhsT=e_all[:, b, :], rhs=ones_col[:], start=True, stop=True
            )
            s16 = smpool.tile([NT, 1], F32, tag="s16")
            nc.vector.tensor_copy(s16[:], ps16[:])
            ps1 = psum_sm.tile([1, 1], F32, tag="sm")
            nc.tensor.matmul(
                ps1[:], lhsT=s16[:], rhs=ones_col[:NT, :], start=True, stop=True
            )
            s1 = smpool.tile([1, 1], F32, tag="s1")
            nc.vector.tensor_copy(s1[:], ps1[:])
            nc.vector.reciprocal(recip_all[:, b : b + 1], s1[:])

            # broadcast recip across partitions: [128, 1]
            ps128 = psum_sm.tile([P, 1], F32, tag="sm")
            nc.tensor.matmul(
                ps128[:], lhsT=ones_row[:], rhs=recip_all[:, b : b + 1],
                start=True, stop=True,
            )
            r128 = smpool.tile([P, 1], F32, tag="r128")
            nc.vector.tensor_copy(r128[:], ps128[:])

            # --- attn output: normalize, transpose to [16, 128], store ---
            at = smpool.tile([P, NT], F32, tag="at")
            nc.vector.tensor_scalar_mul(at[:], e_all[:, b, :], r128[:])
            psT = psum_sm.tile([NT, P], F32, tag="sm")
            nc.tensor.transpose(psT[:], at[:], ident[:])
            atT = smpool.tile([NT, P], F32, tag="atT")
            nc.vector.tensor_copy(atT[:], psT[:])
            nc.gpsimd.dma_start(
                out=attn_out[b, 0, :].rearrange("(t p) -> t p", p=P), in_=atT[:]
            )

            # --- context: accumulate unnormalized E @ value in PSUM ---
            cps = psum_ctx.tile([1, H], F32)
            for d in range(ND):
                vt = vpool.tile([P, KW, H], F32)
                nc.scalar.dma_start(
                    out=vt[:],
                    in_=value[b, d * KW * P : (d + 1) * KW * P, :].rearrange(
                        "(w p) h -> p w h", p=P
                    ),
                )
                for j in range(KW):
                    t = KW * d + j
                    for c in range(2):
                        nc.tensor.matmul(
                            cps[:, c * 512 : (c + 1) * 512],
                            lhsT=e_all[:, b, t : t + 1],
                            rhs=vt[:, j, c * 512 : (c + 1) * 512],
                            start=(t == 0),
                            stop=(t == NT - 1),
                        )
            csb = smpool.tile([1, H], F32, tag="csb")
            nc.vector.tensor_scalar_mul(csb[:], cps[:], recip_all[:, b : b + 1])
            nc.gpsimd.dma_start(out=ctx_out[b : b + 1, 0, :], in_=csb[:])

    return nc


def shard_inputs(query, key, value, mask, Wq, We):
    in_maps = []
    for i in range(N_CORES):
        s = slice(i * B, (i + 1) * B)
        in_maps.append(
            {
                "query": np.ascontiguousarray(query[s], dtype=np.float32),
                "key": np.ascontiguousarray(key[s], dtype=np.float32),
                "value": np.ascontiguousarray(value[s], dtype=np.float32),
                "mask": np.ascontiguousarray(mask[s], dtype=np.int32),
                "Wq": np.ascontiguousarray(Wq, dtype=np.float32),
                "We": np.ascontiguousarray(We, dtype=np.float32),
            }
        )
    return in_maps


def run(inputs, trace=False, **kwargs):
    """Build, run on 8 cores, gather. Returns (context, attn, BassKernelResults)."""
    from concourse import bass_utils

    nc = build_bass()
    if not nc.is_finalized():
        nc.finalize()
    in_maps = shard_inputs(**inputs)
    res = bass_utils.run_bass_kernel_spmd(
        nc, in_maps, core_ids=list(range(N_CORES)), trace=trace, **kwargs
    )
    ctx = np.concatenate([r["context"] for r in res.results], axis=0)
    attn = np.concatenate([r["attn"] for r in res.results], axis=0)
    return ctx, attn, res


def kernel(**inputs):
    ctx, attn, _ = run(inputs, trace=False)
    return ctx, attn


if __name__ == "__main__":
    rng = np.random.default_rng(0)
    inputs = {
        "query": rng.standard_normal((B_TOTAL, 1, H), dtype=np.float32),
        "key": rng.standard_normal((B_TOTAL, KLEN, H), dtype=np.float32),
        "value": rng.standard_normal((B_TOTAL, KLEN, H), dtype=np.float32),
        "mask": np.ones((B_TOTAL, 1, KLEN), dtype=np.int32),
        "Wq": rng.standard_normal((H, H), dtype=np.float32) / np.sqrt(H),
        "We": rng.standard_normal((H,), dtype=np.float32) / np.sqrt(H),
    }
    c, a, _ = run(inputs)
    print("context", c.shape, "attn", a.shape)


# revision 25
# speedup vs baseline: 1.1942x; 1.1942x over previous
"""Bahdanau attention TRN2 Bass kernel.

Full inputs (B=64) are sharded batch-wise across 8 NeuronCores (8 batches
per core); params (Wq, We) are replicated. Each core computes, for its
batches b:
    q_proj = query[b] @ Wq^T                         # [1, 1024]
    energy[k] = sum_h We[h] * tanh(q_proj[h] + key[b,k,h])
    attn = softmax(energy masked)                    # [1, 2048]
    context = attn @ value[b]                        # [1, 1024]

Per-core layout: keys/values stream as [128(k) x 2 x 1024(h)] double tiles
(1 MiB per DMA; keys on the sync HWDGE ring, values on the scalar ring).
 - DVE: X = key + q_bcast (tensor_add), then fused multiply+accumulate
   (scalar_tensor_tensor) of tanh(X) * We_bcast -> energy column [128,1].
 - ACT: tanh / exp.
 - PE : q-projection matmuls, q/recip partition broadcasts, cross-partition
   sums (matmul with ones), attn transpose, context matvec PSUM accumulation.
Softmax skips max-subtraction: energy = sum_h We[h]*tanh(.) with
|We|~N(0,1/H) is O(1), far from fp32 exp overflow.
"""

import os
import sys

import numpy as np

for _p in ("/opt/trn_rl_repo", "/root/.axon_site/_ro/trn_rl_repo"):
    if os.path.isdir(_p) and _p not in sys.path:
        sys.path.insert(0, _p)

import concourse.bacc as bacc  # noqa: E402
import concourse.bass as bass  # noqa: E402
import concourse.tile as tile  # noqa: E402
from concourse import masks, mybir  # noqa: E402
from contextlib import ExitStack  # noqa: E402

F32 = mybir.dt.float32
I32 = mybir.dt.int32
AF = mybir.ActivationFunctionType
OP = mybir.AluOpType

N_CORES = 8
B_TOTAL = 64
B = B_TOTAL // N_CORES  # batches per core
KLEN = 2048
H = 1024
P = 128
NT = KLEN // P  # 16 key/value tiles per batch
KW = 2  # k-tiles per DMA (1 MiB transfers)
ND = NT // KW  # DMAs per batch per stream
NEG_BIG = 1.0e30


def build_bass():
    nc = bacc.Bacc(name="bahdanau")

    query = nc.dram_tensor("query", [B, 1, H], F32, kind="ExternalInput").ap()
    key = nc.dram_tensor("key", [B, KLEN, H], F32, kind="ExternalInput").ap()
    value = nc.dram_tensor("value", [B, KLEN, H], F32, kind="ExternalInput").ap()
    mask = nc.dram_tensor("mask", [B, 1, KLEN], I32, kind="ExternalInput").ap()
    Wq = nc.dram_tensor("Wq", [H, H], F32, kind="ExternalInput").ap()
    We = nc.dram_tensor("We", [H], F32, kind="ExternalInput").ap()
    ctx_out = nc.dram_tensor("context", [B, 1, H], F32, kind="ExternalOutput").ap()
    attn_out = nc.dram_tensor("attn", [B, 1, KLEN], F32, kind="ExternalOutput").ap()

    with tile.TileContext(nc) as tc, ExitStack() as ctx:
        singles = ctx.enter_context(tc.tile_pool(name="singles", bufs=1))

        ident = singles.tile([P, P], F32)
        masks.make_identity(nc, ident[:])
        ones_col = singles.tile([P, 1], F32)
        nc.vector.memset(ones_col[:], 1.0)
        ones_row = singles.tile([1, P], F32)
        nc.vector.memset(ones_row[:], 1.0)

        # We broadcast across partitions: [128, 1024]
        we_b = singles.tile([P, H], F32)
        nc.gpsimd.dma_start(out=we_b[:], in_=We.partition_broadcast(P))

        # Persistent per-batch results
        energy = singles.tile([P, B, NT], F32)
        e_all = singles.tile([P, B, NT], F32)  # exp(masked energy)
        recip_all = singles.tile([1, B], F32)  # 1/sum per batch
        q_sb = singles.tile([B, H], F32)  # q_proj rows

        # ---- main pools (allocated below setup pools on the SBUF stack so
        # the batch loop's DMAs don't serialize on the setup pools' release)
        kpool = ctx.enter_context(tc.tile_pool(name="kpool", bufs=3))
        vpool = ctx.enter_context(tc.tile_pool(name="vpool", bufs=3))
        tpool = ctx.enter_context(tc.tile_pool(name="tpool", bufs=2))
        smpool = ctx.enter_context(tc.tile_pool(name="smpool", bufs=2))
        q_bc_all = singles.tile([P, B, H], F32)  # q_proj broadcast per batch
        psum_sm = ctx.enter_context(tc.tile_pool(name="psum_sm", bufs=1, space="PSUM"))
        psum_ctx = ctx.enter_context(
            tc.tile_pool(name="psum_ctx", bufs=1, space="PSUM")
        )

        # ---- setup: q_proj = query @ Wq^T (scoped pools, freed after) ----
        with ExitStack() as setup:
            sp = setup.enter_context(tc.tile_pool(name="setup_sb", bufs=1))
            wqt_pool = setup.enter_context(tc.tile_pool(name="setup_wqt", bufs=2))
            spsum = setup.enter_context(
                tc.tile_pool(name="setup_ps", bufs=3, space="PSUM")
            )
            qps_pool = setup.enter_context(
                tc.tile_pool(name="setup_qps", bufs=1, space="PSUM")
            )

            # load query rows [8, 1024] and transpose to qT [128(d), 64]
            q_nat = sp.tile([B, H], F32)
            nc.sync.dma_start(out=q_nat[:], in_=query[:, 0, :])
            qT = sp.tile([P, B * (H // P)], F32)  # block i at cols [i*8,(i+1)*8)
            for i in range(H // P):
                pt = spsum.tile([P, B], F32, tag="tp")
                nc.tensor.transpose(pt[:], q_nat[:, i * P : (i + 1) * P], ident[:B, :B])
                nc.vector.tensor_copy(qT[:, i * B : (i + 1) * B], pt[:])

            # load Wq natural tiles [128(h'), j, 1024(d)]
            wq_nat = sp.tile([P, H // P, H], F32)
            nc.sync.dma_start(
                out=wq_nat[:],
                in_=Wq.rearrange("(j p) d -> p j d", p=P),
            )
            # interleave per-d-block transposes with the q_proj accumulation
            # so the PE chain is transpose(i) -> matmul(i) -> transpose(i+1)...
            q_ps_tile = qps_pool.tile([B, H], F32, tag="qps")
            q_ps = [q_ps_tile[:, c * 512 : (c + 1) * 512] for c in range(2)]
            for i in range(H // P):
                wqT_i = wqt_pool.tile([P, H], F32)  # [128(d), 1024(h')]
                for j in range(H // P):
                    pt = spsum.tile([P, P], F32, tag="tp")
                    nc.tensor.transpose(
                        pt[:], wq_nat[:, j, i * P : (i + 1) * P], ident[:]
                    )
                    nc.vector.tensor_copy(wqT_i[:, j * P : (j + 1) * P], pt[:])
                for c in range(2):
                    nc.tensor.matmul(
                        q_ps[c],
                        lhsT=qT[:, i * B : (i + 1) * B],
                        rhs=wqT_i[:, c * 512 : (c + 1) * 512],
                        start=(i == 0),
                        stop=(i == H // P - 1),
                        skip_group_check=True,
                    )
            for c in range(2):
                nc.scalar.copy(q_sb[:, c * 512 : (c + 1) * 512], q_ps[c])

        # broadcast every batch's q_proj row across 128 partitions upfront so
        # the per-batch adds never wait behind context matmuls on PE.
        # (rows first moved to a partition-0 tile; matmul operands must be
        # based at partition 0/32/64)
        for b in range(B):
            q_row = smpool.tile([1, H], F32, tag="qrow")
            nc.gpsimd.dma_start(out=q_row[:], in_=q_sb[b : b + 1, :])
            for c in range(2):
                qps = psum_sm.tile([P, 512], F32, tag="sm")
                nc.tensor.matmul(
                    qps[:],
                    lhsT=ones_row[:],
                    rhs=q_row[:, c * 512 : (c + 1) * 512],
                    start=True,
                    stop=True,
                )
                nc.scalar.copy(q_bc_all[:, b, c * 512 : (c + 1) * 512], qps[:])

        for b in range(B):
            q_bc = q_bc_all[:, b, :]
            # --- energies over key tiles (KW k-tiles per 2 MiB DMA) ---
            for d in range(ND):
                kt = kpool.tile([P, KW, H], F32)
                nc.sync.dma_start(
                    out=kt[:],
                    in_=key[b, d * KW * P : (d + 1) * KW * P, :].rearrange(
                        "(w p) h -> p w h", p=P
                    ),
                )
                for j in range(KW):
                    nc.vector.tensor_add(kt[:, j, :], kt[:, j, :], q_bc)
                tt = tpool.tile([P, KW, H], F32)
                nc.scalar.activation(tt[:], kt[:], AF.Tanh)
                for j in range(KW):
                    t = KW * d + j
                    nc.vector.scalar_tensor_tensor(
                        out=tt[:, j, :],
                        in0=tt[:, j, :],
                        scalar=1.0,
                        in1=we_b[:],
                        op0=OP.mult,
                        op1=OP.mult,
                        accum_out=energy[:, b, t : t + 1],
                    )

            # --- mask + exp ---
            mt_i = smpool.tile([P, NT], I32, tag="mi")
            nc.gpsimd.dma_start(
                out=mt_i[:], in_=mask[b, 0, :].rearrange("(t p) -> p t", p=P)
            )
            mt_f = smpool.tile([P, NT], F32, tag="mf")
            nc.vector.tensor_copy(mt_f[:], mt_i[:])
            pen = smpool.tile([P, NT], F32, tag="pen")
            # (mask - 1) * BIG : 0 where mask==1, -BIG where mask==0
            nc.vector.tensor_scalar(
                out=pen[:], in0=mt_f[:], scalar1=-1.0, scalar2=NEG_BIG,
                op0=OP.add, op1=OP.mult,
            )
            me = smpool.tile([P, NT], F32, tag="me")
            nc.vector.tensor_mul(me[:], energy[:, b, :], mt_f[:])
            nc.vector.tensor_add(me[:], me[:], pen[:])
            nc.scalar.activation(e_all[:, b, :], me[:], AF.Exp)

            # --- softmax denominator via PE column sums ---
            ps16 = psum_sm.tile([NT, 1], F32, tag="sm")
            nc.tensor.matmul(
                ps16[:], lhsT=e_all[:, b, :], rhs=ones_col[:], start=True, stop=True
            )
            s16 = smpool.tile([NT, 1], F32, tag="s16")
            nc.vector.tensor_copy(s16[:], ps16[:])
            ps1 = psum_sm.tile([1, 1], F32, tag="sm")
            nc.tensor.matmul(
                ps1[:], lhsT=s16[:], rhs=ones_col[:NT, :], start=True, stop=True
            )
            s1 = smpool.tile([1, 1], F32, tag="s1")
            nc.vector.tensor_copy(s1[:], ps1[:])
            nc.vector.reciprocal(recip_all[:, b : b + 1], s1[:])

            # broadcast recip across partitions: [128, 1]
            ps128 = psum_sm.tile([P, 1], F32, tag="sm")
            nc.tensor.matmul(
                ps128[:], lhsT=ones_row[:], rhs=recip_all[:, b : b + 1],
                start=True, stop=True,
            )
            r128 = smpool.tile([P, 1], F32, tag="r128")
            nc.vector.tensor_copy(r128[:], ps128[:])

            # --- attn output: normalize, transpose to [16, 128], store ---
            at = smpool.tile([P, NT], F32, tag="at")
            nc.vector.tensor_scalar_mul(at[:], e_all[:, b, :], r128[:])
            psT = psum_sm.tile([NT, P], F32, tag="sm")
            nc.tensor.transpose(psT[:], at[:], ident[:])
            atT = smpool.tile([NT, P], F32, tag="atT")
            nc.vector.tensor_copy(atT[:], psT[:])
            nc.gpsimd.dma_start(
                out=attn_out[b, 0, :].rearrange("(t p) -> t p", p=P), in_=atT[:]
            )

            # --- context: accumulate unnormalized E @ value in PSUM ---
            cps = psum_ctx.tile([1, H], F32)
            for d in range(ND):
                vt = vpool.tile([P, KW, H], F32)
                nc.scalar.dma_start(
                    out=vt[:],
                    in_=value[b, d * KW * P : (d + 1) * KW * P, :].rearrange(
                        "(w p) h -> p w h", p=P
                    ),
                )
                for j in range(KW):
                    t = KW * d + j
                    for c in range(2):
                        nc.tensor.matmul(
                            cps[:, c * 512 : (c + 1) * 512],
                            lhsT=e_all[:, b, t : t + 1],
                            rhs=vt[:, j, c * 512 : (c + 1) * 512],
                            start=(t == 0),
                            stop=(t == NT - 1),
                        )
            csb = smpool.tile([1, H], F32, tag="csb")
            nc.vector.tensor_scalar_mul(csb[:], cps[:], recip_all[:, b : b + 1])
            nc.gpsimd.dma_start(out=ctx_out[b : b + 1, 0, :], in_=csb[:])

    return nc


def shard_inputs(query, key, value, mask, Wq, We):
    in_maps = []
    for i in range(N_CORES):
        s = slice(i * B, (i + 1) * B)
        in_maps.append(
            {
                "query": np.ascontiguousarray(query[s], dtype=np.float32),
                "key": np.ascontiguousarray(key[s], dtype=np.float32),
                "value": np.ascontiguousarray(value[s], dtype=np.float32),
                "mask": np.ascontiguousarray(mask[s], dtype=np.int32),
                "Wq": np.ascontiguousarray(Wq, dtype=np.float32),
                "We": np.ascontiguousarray(We, dtype=np.float32),
            }
        )
    return in_maps


def run(inputs, trace=False, **kwargs):
    """Build, run on 8 cores, gather. Returns (context, attn, BassKernelResults)."""
    from concourse import bass_utils

    nc = build_bass()
    if not nc.is_finalized():
        nc.finalize()
    in_maps = shard_inputs(**inputs)
    res = bass_utils.run_bass_kernel_spmd(
        nc, in_maps, core_ids=list(range(N_CORES)), trace=trace, **kwargs
    )
    ctx = np.concatenate([r["context"] for r in res.results], axis=0)
    attn = np.concatenate([r["attn"] for r in res.results], axis=0)
    return ctx, attn, res


def kernel(**inputs):
    ctx, attn, _ = run(inputs, trace=False)
    return ctx, attn


if __name__ == "__main__":
    rng = np.random.default_rng(0)
    inputs = {
        "query": rng.standard_normal((B_TOTAL, 1, H), dtype=np.float32),
        "key": rng.standard_normal((B_TOTAL, KLEN, H), dtype=np.float32),
        "value": rng.standard_normal((B_TOTAL, KLEN, H), dtype=np.float32),
        "mask": np.ones((B_TOTAL, 1, KLEN), dtype=np.int32),
        "Wq": rng.standard_normal((H, H), dtype=np.float32) / np.sqrt(H),
        "We": rng.standard_normal((H,), dtype=np.float32) / np.sqrt(H),
    }
    c, a, _ = run(inputs)
    print("context", c.shape, "attn", a.shape)


# revision 28
# speedup vs baseline: 1.2217x; 1.0230x over previous
"""Bahdanau attention TRN2 Bass kernel.

Full inputs (B=64) are sharded batch-wise across 8 NeuronCores (8 batches
per core); params (Wq, We) are replicated. Each core computes, for its
batches b:
    q_proj = query[b] @ Wq^T                         # [1, 1024]
    energy[k] = sum_h We[h] * tanh(q_proj[h] + key[b,k,h])
    attn = softmax(energy masked)                    # [1, 2048]
    context = attn @ value[b]                        # [1, 1024]

Per-core layout: keys/values stream as [128(k) x 2 x 1024(h)] double tiles
(1 MiB per DMA; keys on the sync HWDGE ring, values on the scalar ring).
 - DVE: X = key + q_bcast (tensor_add), then fused multiply+accumulate
   (scalar_tensor_tensor) of tanh(X) * We_bcast -> energy column [128,1].
 - ACT: tanh / exp.
 - PE : q-projection matmuls, q/recip partition broadcasts, cross-partition
   sums (matmul with ones), attn transpose, context matvec PSUM accumulation.
Softmax skips max-subtraction: energy = sum_h We[h]*tanh(.) with
|We|~N(0,1/H) is O(1), far from fp32 exp overflow.
"""

import os
import sys

import numpy as np

for _p in ("/opt/trn_rl_repo", "/root/.axon_site/_ro/trn_rl_repo"):
    if os.path.isdir(_p) and _p not in sys.path:
        sys.path.insert(0, _p)

import concourse.bacc as bacc  # noqa: E402
import concourse.bass as bass  # noqa: E402
import concourse.tile as tile  # noqa: E402
from concourse import masks, mybir  # noqa: E402
from contextlib import ExitStack  # noqa: E402

F32 = mybir.dt.float32
I32 = mybir.dt.int32
AF = mybir.ActivationFunctionType
OP = mybir.AluOpType

N_CORES = 8
B_TOTAL = 64
B = B_TOTAL // N_CORES  # batches per core
KLEN = 2048
H = 1024
P = 128
NT = KLEN // P  # 16 key/value tiles per batch
KW = 2  # k-tiles per DMA (1 MiB transfers)
ND = NT // KW  # DMAs per batch per stream
NEG_BIG = 1.0e30


def build_bass():
    nc = bacc.Bacc(name="bahdanau")

    query = nc.dram_tensor("query", [B, 1, H], F32, kind="ExternalInput").ap()
    key = nc.dram_tensor("key", [B, KLEN, H], F32, kind="ExternalInput").ap()
    value = nc.dram_tensor("value", [B, KLEN, H], F32, kind="ExternalInput").ap()
    mask = nc.dram_tensor("mask", [B, 1, KLEN], I32, kind="ExternalInput").ap()
    Wq = nc.dram_tensor("Wq", [H, H], F32, kind="ExternalInput").ap()
    We = nc.dram_tensor("We", [H], F32, kind="ExternalInput").ap()
    ctx_out = nc.dram_tensor("context", [B, 1, H], F32, kind="ExternalOutput").ap()
    attn_out = nc.dram_tensor("attn", [B, 1, KLEN], F32, kind="ExternalOutput").ap()

    with tile.TileContext(nc) as tc, ExitStack() as ctx:
        singles = ctx.enter_context(tc.tile_pool(name="singles", bufs=1))
        dram = ctx.enter_context(tc.tile_pool(name="dram", bufs=1, space="DRAM"))

        ident = singles.tile([P, P], F32)
        masks.make_identity(nc, ident[:])
        ones_col = singles.tile([P, 1], F32)
        nc.vector.memset(ones_col[:], 1.0)
        ones_row = singles.tile([1, P], F32)
        nc.vector.memset(ones_row[:], 1.0)

        # We broadcast across partitions: [128, 1024]
        we_b = singles.tile([P, H], F32)
        nc.gpsimd.dma_start(out=we_b[:], in_=We.partition_broadcast(P))

        # Persistent per-batch results
        energy = singles.tile([P, B, NT], F32)
        e_all = singles.tile([P, B, NT], F32)  # exp(masked energy)
        recip_all = singles.tile([1, B], F32)  # 1/sum per batch
        q_sb = singles.tile([B, H], F32)  # q_proj rows

        # ---- main pools (allocated below setup pools on the SBUF stack so
        # the batch loop's DMAs don't serialize on the setup pools' release)
        kpool = ctx.enter_context(tc.tile_pool(name="kpool", bufs=3))
        vpool = ctx.enter_context(tc.tile_pool(name="vpool", bufs=3))
        tpool = ctx.enter_context(tc.tile_pool(name="tpool", bufs=2))
        smpool = ctx.enter_context(tc.tile_pool(name="smpool", bufs=2))
        q_bc_all = singles.tile([P, B, H], F32)  # q_proj broadcast per batch
        psum_sm = ctx.enter_context(tc.tile_pool(name="psum_sm", bufs=1, space="PSUM"))
        psum_ctx = ctx.enter_context(
            tc.tile_pool(name="psum_ctx", bufs=1, space="PSUM")
        )

        # ---- setup: q_proj = query @ Wq^T (scoped pools, freed after) ----
        with ExitStack() as setup:
            sp = setup.enter_context(tc.tile_pool(name="setup_sb", bufs=1))
            wqt_pool = setup.enter_context(tc.tile_pool(name="setup_wqt", bufs=2))
            spsum = setup.enter_context(
                tc.tile_pool(name="setup_ps", bufs=3, space="PSUM")
            )
            qps_pool = setup.enter_context(
                tc.tile_pool(name="setup_qps", bufs=1, space="PSUM")
            )

            # load query rows [8, 1024] and transpose to qT [128(d), 64]
            q_nat = sp.tile([B, H], F32)
            nc.sync.dma_start(out=q_nat[:], in_=query[:, 0, :])
            qT = sp.tile([P, B * (H // P)], F32)  # block i at cols [i*8,(i+1)*8)
            for i in range(H // P):
                pt = spsum.tile([P, B], F32, tag="tp")
                nc.tensor.transpose(pt[:], q_nat[:, i * P : (i + 1) * P], ident[:B, :B])
                nc.vector.tensor_copy(qT[:, i * B : (i + 1) * B], pt[:])

            # load Wq natural tiles [128(h'), j, 1024(d)] — split per j so the
            # first transposes start before the whole 4 MiB lands
            wq_nat = sp.tile([P, H // P, H], F32)
            wq_re = Wq.rearrange("(j p) d -> p j d", p=P)
            for j in range(H // P):
                nc.sync.dma_start(out=wq_nat[:, j, :], in_=wq_re[:, j, :])
            # interleave per-d-block transposes with the q_proj accumulation
            # so the PE chain is transpose(i) -> matmul(i) -> transpose(i+1)...
            q_ps_tile = qps_pool.tile([B, H], F32, tag="qps")
            q_ps = [q_ps_tile[:, c * 512 : (c + 1) * 512] for c in range(2)]
            for i in range(H // P):
                wqT_i = wqt_pool.tile([P, H], F32)  # [128(d), 1024(h')]
                for j in range(H // P):
                    pt = spsum.tile([P, P], F32, tag="tp")
                    nc.tensor.transpose(
                        pt[:], wq_nat[:, j, i * P : (i + 1) * P], ident[:]
                    )
                    nc.vector.tensor_copy(wqT_i[:, j * P : (j + 1) * P], pt[:])
                for c in range(2):
                    nc.tensor.matmul(
                        q_ps[c],
                        lhsT=qT[:, i * B : (i + 1) * B],
                        rhs=wqT_i[:, c * 512 : (c + 1) * 512],
                        start=(i == 0),
                        stop=(i == H // P - 1),
                        skip_group_check=True,
                    )
            for c in range(2):
                nc.scalar.copy(q_sb[:, c * 512 : (c + 1) * 512], q_ps[c])

        # broadcast every batch's q_proj row across 128 partitions upfront via
        # a DRAM round trip (partition-broadcast DMA needs a DRAM source).
        # Two shots so batch 0's slice is ready early.
        q_dram = dram.tile([B, H], F32)
        nc.gpsimd.dma_start(out=q_dram[:], in_=q_sb[:])
        for b0, b1 in ((0, 2), (2, B)):
            nc.gpsimd.dma_start(
                out=q_bc_all[:, b0:b1, :],
                in_=q_dram[b0:b1, :].partition_broadcast(P),
            )

        for b in range(B):
            q_bc = q_bc_all[:, b, :]
            # --- energies over key tiles (KW k-tiles per 2 MiB DMA) ---
            for d in range(ND):
                kt = kpool.tile([P, KW, H], F32)
                nc.sync.dma_start(
                    out=kt[:],
                    in_=key[b, d * KW * P : (d + 1) * KW * P, :].rearrange(
                        "(w p) h -> p w h", p=P
                    ),
                )
                for j in range(KW):
                    nc.vector.tensor_add(kt[:, j, :], kt[:, j, :], q_bc)
                tt = tpool.tile([P, KW, H], F32)
                nc.scalar.activation(tt[:], kt[:], AF.Tanh)
                for j in range(KW):
                    t = KW * d + j
                    nc.vector.scalar_tensor_tensor(
                        out=tt[:, j, :],
                        in0=tt[:, j, :],
                        scalar=1.0,
                        in1=we_b[:],
                        op0=OP.mult,
                        op1=OP.mult,
                        accum_out=energy[:, b, t : t + 1],
                    )

            # --- mask + exp ---
            mt_i = smpool.tile([P, NT], I32, tag="mi")
            nc.gpsimd.dma_start(
                out=mt_i[:], in_=mask[b, 0, :].rearrange("(t p) -> p t", p=P)
            )
            mt_f = smpool.tile([P, NT], F32, tag="mf")
            nc.vector.tensor_copy(mt_f[:], mt_i[:])
            pen = smpool.tile([P, NT], F32, tag="pen")
            # (mask - 1) * BIG : 0 where mask==1, -BIG where mask==0
            nc.vector.tensor_scalar(
                out=pen[:], in0=mt_f[:], scalar1=-1.0, scalar2=NEG_BIG,
                op0=OP.add, op1=OP.mult,
            )
            me = smpool.tile([P, NT], F32, tag="me")
            nc.vector.tensor_mul(me[:], energy[:, b, :], mt_f[:])
            nc.vector.tensor_add(me[:], me[:], pen[:])
            nc.scalar.activation(e_all[:, b, :], me[:], AF.Exp)

            # --- softmax denominator via PE column sums ---
            ps16 = psum_sm.tile([NT, 1], F32, tag="sm")
            nc.tensor.matmul(
                ps16[:], lhsT=e_all[:, b, :], rhs=ones_col[:], start=True, stop=True
            )
            s16 = smpool.tile([NT, 1], F32, tag="s16")
            nc.vector.tensor_copy(s16[:], ps16[:])
            ps1 = psum_sm.tile([1, 1], F32, tag="sm")
            nc.tensor.matmul(
                ps1[:], lhsT=s16[:], rhs=ones_col[:NT, :], start=True, stop=True
            )
            s1 = smpool.tile([1, 1], F32, tag="s1")
            nc.vector.tensor_copy(s1[:], ps1[:])
            nc.vector.reciprocal(recip_all[:, b : b + 1], s1[:])

            # broadcast recip across partitions: [128, 1]
            ps128 = psum_sm.tile([P, 1], F32, tag="sm")
            nc.tensor.matmul(
                ps128[:], lhsT=ones_row[:], rhs=recip_all[:, b : b + 1],
                start=True, stop=True,
            )
            r128 = smpool.tile([P, 1], F32, tag="r128")
            nc.vector.tensor_copy(r128[:], ps128[:])

            # --- attn output: normalize, transpose to [16, 128], store ---
            at = smpool.tile([P, NT], F32, tag="at")
            nc.vector.tensor_scalar_mul(at[:], e_all[:, b, :], r128[:])
            psT = psum_sm.tile([NT, P], F32, tag="sm")
            nc.tensor.transpose(psT[:], at[:], ident[:])
            atT = smpool.tile([NT, P], F32, tag="atT")
            nc.vector.tensor_copy(atT[:], psT[:])
            nc.gpsimd.dma_start(
                out=attn_out[b, 0, :].rearrange("(t p) -> t p", p=P), in_=atT[:]
            )

            # --- context: accumulate unnormalized E @ value in PSUM ---
            cps = psum_ctx.tile([1, H], F32)
            for d in range(ND):
                vt = vpool.tile([P, KW, H], F32)
                nc.scalar.dma_start(
                    out=vt[:],
                    in_=value[b, d * KW * P : (d + 1) * KW * P, :].rearrange(
                        "(w p) h -> p w h", p=P
                    ),
                )
                for j in range(KW):
                    t = KW * d + j
                    for c in range(2):
                        nc.tensor.matmul(
                            cps[:, c * 512 : (c + 1) * 512],
                            lhsT=e_all[:, b, t : t + 1],
                            rhs=vt[:, j, c * 512 : (c + 1) * 512],
                            start=(t == 0),
                            stop=(t == NT - 1),
                        )
            csb = smpool.tile([1, H], F32, tag="csb")
            nc.vector.tensor_scalar_mul(csb[:], cps[:], recip_all[:, b : b + 1])
            nc.gpsimd.dma_start(out=ctx_out[b : b + 1, 0, :], in_=csb[:])

    return nc


def shard_inputs(query, key, value, mask, Wq, We):
    in_maps = []
    for i in range(N_CORES):
        s = slice(i * B, (i + 1) * B)
        in_maps.append(
            {
                "query": np.ascontiguousarray(query[s], dtype=np.float32),
                "key": np.ascontiguousarray(key[s], dtype=np.float32),
                "value": np.ascontiguousarray(value[s], dtype=np.float32),
                "mask": np.ascontiguousarray(mask[s], dtype=np.int32),
                "Wq": np.ascontiguousarray(Wq, dtype=np.float32),
                "We": np.ascontiguousarray(We, dtype=np.float32),
            }
        )
    return in_maps


def run(inputs, trace=False, **kwargs):
    """Build, run on 8 cores, gather. Returns (context, attn, BassKernelResults)."""
    from concourse import bass_utils

    nc = build_bass()
    if not nc.is_finalized():
        nc.finalize()
    in_maps = shard_inputs(**inputs)
    res = bass_utils.run_bass_kernel_spmd(
        nc, in_maps, core_ids=list(range(N_CORES)), trace=trace, **kwargs
    )
    ctx = np.concatenate([r["context"] for r in res.results], axis=0)
    attn = np.concatenate([r["attn"] for r in res.results], axis=0)
    return ctx, attn, res


def kernel(**inputs):
    ctx, attn, _ = run(inputs, trace=False)
    return ctx, attn


if __name__ == "__main__":
    rng = np.random.default_rng(0)
    inputs = {
        "query": rng.standard_normal((B_TOTAL, 1, H), dtype=np.float32),
        "key": rng.standard_normal((B_TOTAL, KLEN, H), dtype=np.float32),
        "value": rng.standard_normal((B_TOTAL, KLEN, H), dtype=np.float32),
        "mask": np.ones((B_TOTAL, 1, KLEN), dtype=np.int32),
        "Wq": rng.standard_normal((H, H), dtype=np.float32) / np.sqrt(H),
        "We": rng.standard_normal((H,), dtype=np.float32) / np.sqrt(H),
    }
    c, a, _ = run(inputs)
    print("context", c.shape, "attn", a.shape)


# revision 29
# speedup vs baseline: 1.3258x; 1.0851x over previous
"""Bahdanau attention TRN2 Bass kernel.

Full inputs (B=64) are sharded batch-wise across 8 NeuronCores (8 batches
per core); params (Wq, We) are replicated. Each core computes, for its
batches b:
    q_proj = query[b] @ Wq^T                         # [1, 1024]
    energy[k] = sum_h We[h] * tanh(q_proj[h] + key[b,k,h])
    attn = softmax(energy masked)                    # [1, 2048]
    context = attn @ value[b]                        # [1, 1024]

Per-core layout: keys/values stream as [128(k) x 2 x 1024(h)] double tiles
(1 MiB per DMA; keys on the sync HWDGE ring, values on the scalar ring).
 - DVE: X = key + q_bcast (tensor_add), then fused multiply+accumulate
   (scalar_tensor_tensor) of tanh(X) * We_bcast -> energy column [128,1].
 - ACT: tanh / exp.
 - PE : q-projection matmuls, q/recip partition broadcasts, cross-partition
   sums (matmul with ones), attn transpose, context matvec PSUM accumulation.
Softmax skips max-subtraction: energy = sum_h We[h]*tanh(.) with
|We|~N(0,1/H) is O(1), far from fp32 exp overflow.
"""

import os
import sys

import numpy as np

for _p in ("/opt/trn_rl_repo", "/root/.axon_site/_ro/trn_rl_repo"):
    if os.path.isdir(_p) and _p not in sys.path:
        sys.path.insert(0, _p)

import concourse.bacc as bacc  # noqa: E402
import concourse.bass as bass  # noqa: E402
import concourse.tile as tile  # noqa: E402
from concourse import masks, mybir  # noqa: E402
from contextlib import ExitStack  # noqa: E402

F32 = mybir.dt.float32
I32 = mybir.dt.int32
AF = mybir.ActivationFunctionType
OP = mybir.AluOpType

N_CORES = 8
B_TOTAL = 64
B = B_TOTAL // N_CORES  # batches per core
KLEN = 2048
H = 1024
P = 128
NT = KLEN // P  # 16 key/value tiles per batch
KW = 2  # k-tiles per DMA (1 MiB transfers)
ND = NT // KW  # DMAs per batch per stream
NEG_BIG = 1.0e30


def build_bass():
    nc = bacc.Bacc(name="bahdanau")

    query = nc.dram_tensor("query", [B, 1, H], F32, kind="ExternalInput").ap()
    key = nc.dram_tensor("key", [B, KLEN, H], F32, kind="ExternalInput").ap()
    value = nc.dram_tensor("value", [B, KLEN, H], F32, kind="ExternalInput").ap()
    mask = nc.dram_tensor("mask", [B, 1, KLEN], I32, kind="ExternalInput").ap()
    Wq = nc.dram_tensor("Wq", [H, H], F32, kind="ExternalInput").ap()
    We = nc.dram_tensor("We", [H], F32, kind="ExternalInput").ap()
    ctx_out = nc.dram_tensor("context", [B, 1, H], F32, kind="ExternalOutput").ap()
    attn_out = nc.dram_tensor("attn", [B, 1, KLEN], F32, kind="ExternalOutput").ap()

    with tile.TileContext(nc) as tc, ExitStack() as ctx:
        singles = ctx.enter_context(tc.tile_pool(name="singles", bufs=1))
        dram = ctx.enter_context(tc.tile_pool(name="dram", bufs=1, space="DRAM"))

        ident = singles.tile([P, P], F32)
        masks.make_identity(nc, ident[:])
        ones_col = singles.tile([P, 1], F32)
        nc.vector.memset(ones_col[:], 1.0)
        ones_row = singles.tile([1, P], F32)
        nc.vector.memset(ones_row[:], 1.0)

        # We broadcast across partitions: [128, 1024]
        we_b = singles.tile([P, H], F32)
        nc.gpsimd.dma_start(out=we_b[:], in_=We.partition_broadcast(P))

        # Persistent per-batch results
        energy = singles.tile([P, B, NT], F32)
        e_all = singles.tile([P, B, NT], F32)  # exp(masked energy)
        recip_all = singles.tile([1, B], F32)  # 1/sum per batch
        q_sb = singles.tile([B, H], F32)  # q_proj rows

        # ---- main pools (allocated below setup pools on the SBUF stack so
        # the batch loop's DMAs don't serialize on the setup pools' release)
        kpool = ctx.enter_context(tc.tile_pool(name="kpool", bufs=4))
        vpool = ctx.enter_context(tc.tile_pool(name="vpool", bufs=4))
        tpool = ctx.enter_context(tc.tile_pool(name="tpool", bufs=3))
        smpool = ctx.enter_context(tc.tile_pool(name="smpool", bufs=2))
        q_bc_all = singles.tile([P, B, H], F32)  # q_proj broadcast per batch
        psum_sm = ctx.enter_context(tc.tile_pool(name="psum_sm", bufs=1, space="PSUM"))
        psum_ctx = ctx.enter_context(
            tc.tile_pool(name="psum_ctx", bufs=1, space="PSUM")
        )

        # ---- setup: q_proj = query @ Wq^T (scoped pools, freed after) ----
        with ExitStack() as setup:
            sp = setup.enter_context(tc.tile_pool(name="setup_sb", bufs=1))
            wqt_pool = setup.enter_context(tc.tile_pool(name="setup_wqt", bufs=2))
            spsum = setup.enter_context(
                tc.tile_pool(name="setup_ps", bufs=3, space="PSUM")
            )
            qps_pool = setup.enter_context(
                tc.tile_pool(name="setup_qps", bufs=1, space="PSUM")
            )

            # load query rows [8, 1024] and transpose to qT [128(d), 64]
            q_nat = sp.tile([B, H], F32)
            nc.sync.dma_start(out=q_nat[:], in_=query[:, 0, :])
            qT = sp.tile([P, B * (H // P)], F32)  # block i at cols [i*8,(i+1)*8)
            for i in range(H // P):
                pt = spsum.tile([P, B], F32, tag="tp")
                nc.tensor.transpose(pt[:], q_nat[:, i * P : (i + 1) * P], ident[:B, :B])
                nc.vector.tensor_copy(qT[:, i * B : (i + 1) * B], pt[:])

            # load Wq natural tiles [128(h'), j, 1024(d)] — split per j so the
            # first transposes start before the whole 4 MiB lands
            wq_nat = sp.tile([P, H // P, H], F32)
            wq_re = Wq.rearrange("(j p) d -> p j d", p=P)
            for j in range(H // P):
                nc.sync.dma_start(out=wq_nat[:, j, :], in_=wq_re[:, j, :])
            # interleave per-d-block transposes with the q_proj accumulation
            # so the PE chain is transpose(i) -> matmul(i) -> transpose(i+1)...
            q_ps_tile = qps_pool.tile([B, H], F32, tag="qps")
            q_ps = [q_ps_tile[:, c * 512 : (c + 1) * 512] for c in range(2)]
            for i in range(H // P):
                wqT_i = wqt_pool.tile([P, H], F32)  # [128(d), 1024(h')]
                for j in range(H // P):
                    pt = spsum.tile([P, P], F32, tag="tp")
                    nc.tensor.transpose(
                        pt[:], wq_nat[:, j, i * P : (i + 1) * P], ident[:]
                    )
                    nc.vector.tensor_copy(wqT_i[:, j * P : (j + 1) * P], pt[:])
                for c in range(2):
                    nc.tensor.matmul(
                        q_ps[c],
                        lhsT=qT[:, i * B : (i + 1) * B],
                        rhs=wqT_i[:, c * 512 : (c + 1) * 512],
                        start=(i == 0),
                        stop=(i == H // P - 1),
                        skip_group_check=True,
                    )
            for c in range(2):
                nc.scalar.copy(q_sb[:, c * 512 : (c + 1) * 512], q_ps[c])

        # broadcast every batch's q_proj row across 128 partitions upfront via
        # a DRAM round trip (partition-broadcast DMA needs a DRAM source).
        # Two shots so batch 0's slice is ready early.
        q_dram = dram.tile([B, H], F32)
        nc.gpsimd.dma_start(out=q_dram[:], in_=q_sb[:])
        for b0, b1 in ((0, 2), (2, B)):
            nc.gpsimd.dma_start(
                out=q_bc_all[:, b0:b1, :],
                in_=q_dram[b0:b1, :].partition_broadcast(P),
            )

        for b in range(B):
            q_bc = q_bc_all[:, b, :]
            # --- energies over key tiles (KW k-tiles per 2 MiB DMA) ---
            for d in range(ND):
                kt = kpool.tile([P, KW, H], F32)
                nc.sync.dma_start(
                    out=kt[:],
                    in_=key[b, d * KW * P : (d + 1) * KW * P, :].rearrange(
                        "(w p) h -> p w h", p=P
                    ),
                )
                for j in range(KW):
                    nc.vector.tensor_add(kt[:, j, :], kt[:, j, :], q_bc)
                tt = tpool.tile([P, KW, H], F32)
                nc.scalar.activation(tt[:], kt[:], AF.Tanh)
                for j in range(KW):
                    t = KW * d + j
                    nc.vector.scalar_tensor_tensor(
                        out=tt[:, j, :],
                        in0=tt[:, j, :],
                        scalar=1.0,
                        in1=we_b[:],
                        op0=OP.mult,
                        op1=OP.mult,
                        accum_out=energy[:, b, t : t + 1],
                    )

            # --- mask + exp ---
            mt_i = smpool.tile([P, NT], I32, tag="mi")
            nc.gpsimd.dma_start(
                out=mt_i[:], in_=mask[b, 0, :].rearrange("(t p) -> p t", p=P)
            )
            mt_f = smpool.tile([P, NT], F32, tag="mf")
            nc.vector.tensor_copy(mt_f[:], mt_i[:])
            pen = smpool.tile([P, NT], F32, tag="pen")
            # (mask - 1) * BIG : 0 where mask==1, -BIG where mask==0
            nc.vector.tensor_scalar(
                out=pen[:], in0=mt_f[:], scalar1=-1.0, scalar2=NEG_BIG,
                op0=OP.add, op1=OP.mult,
            )
            me = smpool.tile([P, NT], F32, tag="me")
            nc.vector.tensor_mul(me[:], energy[:, b, :], mt_f[:])
            nc.vector.tensor_add(me[:], me[:], pen[:])
            nc.scalar.activation(e_all[:, b, :], me[:], AF.Exp)

            # --- context: accumulate unnormalized E @ value in PSUM ---
            # (emitted before the softmax-denominator smalls so the PE FIFO
            # reaches the value matmuls without stalling on DVE copies)
            cps = psum_ctx.tile([1, H], F32)
            for d in range(ND):
                vt = vpool.tile([P, KW, H], F32)
                nc.scalar.dma_start(
                    out=vt[:],
                    in_=value[b, d * KW * P : (d + 1) * KW * P, :].rearrange(
                        "(w p) h -> p w h", p=P
                    ),
                )
                for j in range(KW):
                    t = KW * d + j
                    for c in range(2):
                        nc.tensor.matmul(
                            cps[:, c * 512 : (c + 1) * 512],
                            lhsT=e_all[:, b, t : t + 1],
                            rhs=vt[:, j, c * 512 : (c + 1) * 512],
                            start=(t == 0),
                            stop=(t == NT - 1),
                        )
            # --- softmax denominator via PE column sums ---
            ps16 = psum_sm.tile([NT, 1], F32, tag="sm")
            nc.tensor.matmul(
                ps16[:], lhsT=e_all[:, b, :], rhs=ones_col[:], start=True, stop=True
            )
            s16 = smpool.tile([NT, 1], F32, tag="s16")
            nc.vector.tensor_copy(s16[:], ps16[:])
            ps1 = psum_sm.tile([1, 1], F32, tag="sm")
            nc.tensor.matmul(
                ps1[:], lhsT=s16[:], rhs=ones_col[:NT, :], start=True, stop=True
            )
            s1 = smpool.tile([1, 1], F32, tag="s1")
            nc.vector.tensor_copy(s1[:], ps1[:])
            nc.vector.reciprocal(recip_all[:, b : b + 1], s1[:])

            # broadcast recip across partitions: [128, 1]
            ps128 = psum_sm.tile([P, 1], F32, tag="sm")
            nc.tensor.matmul(
                ps128[:], lhsT=ones_row[:], rhs=recip_all[:, b : b + 1],
                start=True, stop=True,
            )
            r128 = smpool.tile([P, 1], F32, tag="r128")
            nc.vector.tensor_copy(r128[:], ps128[:])

            # --- attn output: normalize, transpose to [16, 128], store ---
            at = smpool.tile([P, NT], F32, tag="at")
            nc.vector.tensor_scalar_mul(at[:], e_all[:, b, :], r128[:])
            psT = psum_sm.tile([NT, P], F32, tag="sm")
            nc.tensor.transpose(psT[:], at[:], ident[:])
            atT = smpool.tile([NT, P], F32, tag="atT")
            nc.vector.tensor_copy(atT[:], psT[:])
            nc.gpsimd.dma_start(
                out=attn_out[b, 0, :].rearrange("(t p) -> t p", p=P), in_=atT[:]
            )

            csb = smpool.tile([1, H], F32, tag="csb")
            nc.vector.tensor_scalar_mul(csb[:], cps[:], recip_all[:, b : b + 1])
            nc.gpsimd.dma_start(out=ctx_out[b : b + 1, 0, :], in_=csb[:])

    return nc


def shard_inputs(query, key, value, mask, Wq, We):
    in_maps = []
    for i in range(N_CORES):
        s = slice(i * B, (i + 1) * B)
        in_maps.append(
            {
                "query": np.ascontiguousarray(query[s], dtype=np.float32),
                "key": np.ascontiguousarray(key[s], dtype=np.float32),
                "value": np.ascontiguousarray(value[s], dtype=np.float32),
                "mask": np.ascontiguousarray(mask[s], dtype=np.int32),
                "Wq": np.ascontiguousarray(Wq, dtype=np.float32),
                "We": np.ascontiguousarray(We, dtype=np.float32),
            }
        )
    return in_maps


def run(inputs, trace=False, **kwargs):
    """Build, run on 8 cores, gather. Returns (context, attn, BassKernelResults)."""
    from concourse import bass_utils

    nc = build_bass()
    if not nc.is_finalized():
        nc.finalize()
    in_maps = shard_inputs(**inputs)
    res = bass_utils.run_bass_kernel_spmd(
        nc, in_maps, core_ids=list(range(N_CORES)), trace=trace, **kwargs
    )
    ctx = np.concatenate([r["context"] for r in res.results], axis=0)
    attn = np.concatenate([r["attn"] for r in res.results], axis=0)
    return ctx, attn, res


def kernel(**inputs):
    ctx, attn, _ = run(inputs, trace=False)
    return ctx, attn


if __name__ == "__main__":
    rng = np.random.default_rng(0)
    inputs = {
        "query": rng.standard_normal((B_TOTAL, 1, H), dtype=np.float32),
        "key": rng.standard_normal((B_TOTAL, KLEN, H), dtype=np.float32),
        "value": rng.standard_normal((B_TOTAL, KLEN, H), dtype=np.float32),
        "mask": np.ones((B_TOTAL, 1, KLEN), dtype=np.int32),
        "Wq": rng.standard_normal((H, H), dtype=np.float32) / np.sqrt(H),
        "We": rng.standard_normal((H,), dtype=np.float32) / np.sqrt(H),
    }
    c, a, _ = run(inputs)
    print("context", c.shape, "attn", a.shape)
